# revision 36
# baseline (speedup 1.0000x reference)
"""DGCN (diffusion graph conv) Trainium2 Bass kernel.

Reference computation (per batch b, time t):
    h   = relu(st_emb @ W1 + b1)              # [T,1,32]
    lam = 1 + relu(h @ W2 + b2)               # [T,1,1]
    c1  = 2 - 2/lam ;  c2 = 2/lam             # scalars per t
    out[b,t] = c1[t] * (x[b,t] @ W0) + c2[t] * ((adj @ x[b,t]) @ W1g) + bias
where weights = [W0; W1g] with shape [2, 64, 64].

Strategy: data-parallel over batch B=8 across the 8 NeuronCores (adj and
weights replicated).  Per core, with x_b node-major X' [N, T*C] and adjT:
    YT[tc, i] = sum_j X'[j, tc] * adjT[j, i]          (one big fp32r matmul,
                                                       K=2048, M=768, N=2048)
    out pairs: for each pair of timesteps (2m, 2m+1), a block-diagonal
    stationary [128,128] = diag(c*W, c*W) contracts the 128-row
    (two-timestep) channel-major slabs of x and YT in a single matmul,
    accumulating the identity-term and adjacency-term into one PSUM bank.

Everything on-device computes in float32r (TF32-like, 1 cycle/row on the PE)
except the tiny lambda-MLP which runs in full fp32.
"""
import numpy as np

import concourse.bass as bass
import concourse.tile as tile
from concourse import bacc, mybir
from concourse.bass_utils import run_bass_kernel_spmd

# Problem shapes (hardcoded per the harness contract).
B, T, N, C = 8, 12, 2048, 64
TC = T * C                     # 768
P = 128                        # partitions
KT = N // P                    # 16 k tiles
NCHUNK = 512                   # node columns per chunk
CHUNKS = N // NCHUNK           # 4
MT = TC // P                   # 6 tc (pair-of-timestep) tiles
NPAIR = T // 2                 # 6

F32 = mybir.dt.float32
F32R = mybir.dt.float32r


def build_kernel(repeat=1):
    nc = bacc.Bacc(name="dgcn")

    # ---- per-core external inputs -------------------------------------
    xnode = nc.dram_tensor("xnode", [N, TC], F32R, kind="ExternalInput")
    adjt = nc.dram_tensor("adjt", [N, N], F32R, kind="ExternalInput")
    ident = nc.dram_tensor("ident", [P, P], F32R, kind="ExternalInput")
    sT = nc.dram_tensor("sT", [64, T], F32, kind="ExternalInput")        # st_emb.T
    w1 = nc.dram_tensor("w1", [64, 32], F32, kind="ExternalInput")
    b1p = nc.dram_tensor("b1p", [32, 1], F32, kind="ExternalInput")
    w2 = nc.dram_tensor("w2", [32, 1], F32, kind="ExternalInput")
    b2p = nc.dram_tensor("b2p", [1, 1], F32, kind="ExternalInput")
    # block-diagonal weight templates diag(Wk, Wk), [128, 128] each
    wd0 = nc.dram_tensor("wd0", [P, P], F32, kind="ExternalInput")
    wd1 = nc.dram_tensor("wd1", [P, P], F32, kind="ExternalInput")
    biasp = nc.dram_tensor("biasp", [P, 1], F32, kind="ExternalInput")   # bias twice
    masks = nc.dram_tensor("masks", [2, P], F32, kind="ExternalInput")   # upper/lower half sel
    out = nc.dram_tensor("out", [TC, N], F32, kind="ExternalOutput")

    out_ap = out.ap().rearrange("(m p) n -> p m n", p=P)

    with tile.TileContext(nc) as tc:
        with (
            tc.tile_pool(name="const", bufs=1) as const,
            tc.tile_pool(name="xn", bufs=1) as xn_pool,
            tc.tile_pool(name="adj", bufs=5) as adj_pool,
            tc.tile_pool(name="yts", bufs=13) as yts_pool,
            tc.tile_pool(name="xch", bufs=3) as xch_pool,
            tc.tile_pool(name="outs", bufs=4) as outs_pool,
            tc.tile_pool(name="ytps", bufs=1, space="PSUM") as ytps_pool,
            tc.tile_pool(name="miscps", bufs=2, space="PSUM") as misc_ps,
        ):
            # ============ lambda MLP + paired scaled weights =============
            # tiny constants go through SWDGE (gpsimd) so they don't occupy
            # the serial HWDGE dispatch ring; allocated here, loaded after the
            # prologue's big loads are issued (consts aren't needed until the
            # chunk-0 epilogue).
            sT_sb = const.tile([64, T], F32)
            w1_sb = const.tile([64, 32], F32)
            b1_sb = const.tile([32, 1], F32)
            w2_sb = const.tile([32, 1], F32)
            b2_sb = const.tile([1, 1], F32)
            wd0_sb = const.tile([P, P], F32)
            wd1_sb = const.tile([P, P], F32)
            bias_sb = const.tile([P, 1], F32)
            ident_sb = const.tile([P, P], F32R)
            mask_up = const.tile([1, P], F32)
            mask_lo = const.tile([1, P], F32)

            def load_consts():
                nc.gpsimd.dma_start(sT_sb[:], sT.ap())
                nc.gpsimd.dma_start(w1_sb[:], w1.ap())
                nc.gpsimd.dma_start(b1_sb[:], b1p.ap())
                nc.gpsimd.dma_start(w2_sb[:], w2.ap())
                nc.gpsimd.dma_start(b2_sb[:], b2p.ap())
                nc.gpsimd.dma_start(wd0_sb[:], wd0.ap())
                nc.gpsimd.dma_start(wd1_sb[:], wd1.ap())
                nc.gpsimd.dma_start(bias_sb[:], biasp.ap())
                nc.gpsimd.dma_start(ident_sb[:], ident.ap())
                nc.gpsimd.dma_start(mask_up[:], masks.ap()[0:1, :])
                nc.gpsimd.dma_start(mask_lo[:], masks.ap()[1:2, :])

            # scaled block-diagonal stationaries, f32r (filled by emit_mlp)
            wx_sb = const.tile([P, NPAIR, P], F32R)   # identity-term weights
            wy_sb = const.tile([P, NPAIR, P], F32R)   # adjacency-term weights

            def emit_mlp():
                """Lambda MLP + paired scaled weights. Emitted after chunk 0's
                adjacency matmuls so its cross-engine chain doesn't stall the
                PE FIFO while the big input DMAs stream in."""
                # h.T = relu(W1.T @ sT + b1)   [32, T]
                h_ps = misc_ps.tile([P, NCHUNK], F32, tag="mps", name="h_ps")
                nc.tensor.matmul(h_ps[:32, :T], w1_sb[:], sT_sb[:], start=True, stop=True)
                hr_sb = const.tile([32, T], F32)
                nc.scalar.activation(out=hr_sb[:], in_=h_ps[:32, :T],
                                     func=mybir.ActivationFunctionType.Relu,
                                     bias=b1_sb[:], scale=1.0)
                # lam = 1 + relu(W2.T @ hr + b2)   [1, T]
                lam_ps = misc_ps.tile([P, NCHUNK], F32, tag="mps", name="lam_ps")
                nc.tensor.matmul(lam_ps[:1, :T], w2_sb[:], hr_sb[:], start=True, stop=True)
                lam_sb = const.tile([1, T], F32)
                nc.scalar.activation(out=lam_sb[:], in_=lam_ps[:1, :T],
                                     func=mybir.ActivationFunctionType.Relu,
                                     bias=b2_sb[:], scale=1.0)
                lam1_sb = const.tile([1, T], F32)
                nc.vector.tensor_scalar_add(lam1_sb[:], lam_sb[:], 1.0)
                inv_sb = const.tile([1, T], F32)
                nc.vector.reciprocal(out=inv_sb[:], in_=lam1_sb[:])
                c2_sb = const.tile([1, T], F32)
                nc.vector.tensor_scalar_mul(c2_sb[:], inv_sb[:], 2.0)
                c1_sb = const.tile([1, T], F32)
                nc.vector.tensor_scalar(c1_sb[:], inv_sb[:], -2.0, 2.0,
                                        mybir.AluOpType.mult, mybir.AluOpType.add)

                # paired per-partition coefficient columns:
                # cp1[:, m] = [c1[2m]]*64 + [c1[2m+1]]*64, same for cp2.
                cp_ps = misc_ps.tile([P, NCHUNK], F32, tag="mps", name="cp_ps")
                c1_pairs = c1_sb.rearrange("p (a two) -> p two a", two=2)
                c2_pairs = c2_sb.rearrange("p (a two) -> p two a", two=2)
                nc.tensor.matmul(cp_ps[:, :NPAIR], mask_up[:], c1_pairs[:, 0, :],
                                 start=True, stop=False)
                nc.tensor.matmul(cp_ps[:, :NPAIR], mask_lo[:], c1_pairs[:, 1, :],
                                 start=False, stop=False)
                nc.tensor.matmul(cp_ps[:, NPAIR:2 * NPAIR], mask_up[:], c2_pairs[:, 0, :],
                                 start=False, stop=False)
                nc.tensor.matmul(cp_ps[:, NPAIR:2 * NPAIR], mask_lo[:], c2_pairs[:, 1, :],
                                 start=False, stop=True)
                cp_sb = const.tile([P, 2 * NPAIR], F32)
                nc.vector.tensor_copy(out=cp_sb[:], in_=cp_ps[:, :2 * NPAIR])

                for m in range(NPAIR):
                    nc.vector.tensor_scalar_mul(wx_sb[:, m, :], wd0_sb[:], cp_sb[:, m:m + 1])
                    nc.vector.tensor_scalar_mul(wy_sb[:, m, :], wd1_sb[:],
                                                cp_sb[:, NPAIR + m:NPAIR + m + 1])

            # ============ main loop with prefetch pipelining =============
            chunk_seq = [c for _ in range(repeat) for c in range(CHUNKS)]

            adjt_ap = adjt.ap().rearrange("(k p) n -> p k n", p=P)
            xnode_ap = xnode.ap().rearrange("(k p) f -> p k f", p=P)
            KB = 4   # k-tiles per batched (prefetched) DMA

            def load_at(ch, b):
                # one DMA covering k-tiles 4b..4b+3 of this chunk's columns
                at_sb = adj_pool.tile([P, KB, NCHUNK], F32R, tag="at", name="at_sb")
                cs = slice(ch * NCHUNK, (ch + 1) * NCHUNK)
                nc.sync.dma_start(at_sb[:], adjt_ap[:, KB * b:KB * (b + 1), cs])
                return at_sb

            def load_at1(ch, k):
                # single k-tile load (fine-grained, for the first chunk)
                at_sb = adj_pool.tile([P, NCHUNK], F32R, tag="at1", name="at1_sb", bufs=16)
                cs = slice(ch * NCHUNK, (ch + 1) * NCHUNK)
                nc.sync.dma_start(at_sb[:], adjt.ap()[k * P:(k + 1) * P, cs])
                return at_sb

            def alloc_xch():
                return xch_pool.tile([P, MT, NCHUNK], F32R, tag="xch", name="xch_sb")

            def emit_tp(ch, m, xch_sb):
                # derive one channel-major m-slab of chunk ch by PE-transposing
                # the resident node-major tiles (4 transposes batched per bank)
                tp_ps = misc_ps.tile([P, NCHUNK], F32R, tag="mps", name="tp_ps")
                for kk in range(NCHUNK // P):
                    k = ch * (NCHUNK // P) + kk
                    nc.tensor.transpose(tp_ps[:, kk * P:(kk + 1) * P],
                                        xnt_slice(k, m), ident_sb[:])
                nc.vector.tensor_copy(out=xch_sb[:, m, :], in_=tp_ps[:])

            def emit_final_m(ch, m, yts_list, xch_sb, tail=False):
                cs = slice(ch * NCHUNK, (ch + 1) * NCHUNK)
                if tail:
                    # last chunk: the yt accumulator banks are free by now --
                    # use them so all six finals ping-pong without bank waits
                    o_ps = ytps_pool.tile([P, NCHUNK], F32, tag=f"yt{m}",
                                          name="o_ps_t")
                else:
                    o_ps = misc_ps.tile([P, NCHUNK], F32, tag="mps", name="o_ps")
                nc.tensor.matmul(o_ps[:], wx_sb[:, m, :], xch_sb[:, m, :],
                                 start=True, stop=False)
                nc.tensor.matmul(o_ps[:], wy_sb[:, m, :], yts_list[m][:],
                                 start=False, stop=True)
                # bias add while evacuating PSUM; stream out per-m
                out_sb = outs_pool.tile([P, NCHUNK], F32, tag="outsb")
                nc.scalar.activation(out=out_sb[:], in_=o_ps[:],
                                     func=mybir.ActivationFunctionType.Identity,
                                     bias=bias_sb[:], scale=1.0)
                nc.scalar.dma_start(out_ap[:, m, cs], out_sb[:])

            # prologue: fine-grained interleaved loads (one k-tile per DMA)
            # so the first matmul can start ~2us in; later chunks use batched
            # prefetched DMAs to keep HWDGE dispatch count low.
            xnt = []
            cur_at1 = []
            for k in range(KT):
                t_ = xn_pool.tile([P, TC], F32R, tag=f"xnt{k}", name=f"xnt{k}")
                nc.sync.dma_start(t_[:], xnode_ap[:, k, :])
                xnt.append(t_)
                cur_at1.append(load_at1(chunk_seq[0], k))
            load_consts()

            def xnt_slice(k, m):
                return xnt[k][:, m * P:(m + 1) * P]

            cur_xch = None
            cur_at = None    # batched tiles for chunks after the first
            pending = None   # (ch, yts_list, xch_sb) finals woven into next k-loop
            for ci, ch in enumerate(chunk_seq):
                nxt = chunk_seq[ci + 1] if ci + 1 < len(chunk_seq) else None
                nxt_at = []
                nxt_xch = alloc_xch() if nxt is not None else None

                yt_ps = [ytps_pool.tile([P, NCHUNK], F32, tag=f"yt{m}", name=f"yt{m}")
                         for m in range(MT)]
                last = ci == len(chunk_seq) - 1
                khi = KT - 1 if last else KT
                for k in range(khi):
                    if nxt is not None and k % KB == 0:
                        nxt_at.append(load_at(nxt, k // KB))
                    rhs = (cur_at1[k][:] if ci == 0
                           else cur_at[k // KB][:, k % KB, :])
                    for m in range(MT):
                        nc.tensor.matmul(
                            yt_ps[m][:], xnt_slice(k, m), rhs,
                            start=(k == 0), stop=(k == KT - 1),
                        )
                    # weave the previous chunk's finals and the next chunk's
                    # transpose slabs into the adj k-loop: by the time the
                    # in-order PE reaches them their inputs are long ready.
                    if pending is not None and k >= 4 and (k - 4) % 2 == 0:
                        emit_final_m(pending[0], (k - 4) // 2, pending[1], pending[2])
                    if nxt is not None and k >= 5 and (k - 5) % 2 == 0:
                        emit_tp(nxt, (k - 5) // 2, nxt_xch)

                # evacuate PSUM accumulators promptly so the next chunk's
                # accumulation can begin while finals lag one chunk behind.
                # On the last chunk, interleave the final k=15 matmuls with the
                # copies so each accumulator drains as soon as it completes.
                yts_list = []
                if last:
                    k = KT - 1
                    rhs = (cur_at1[k][:] if ci == 0
                           else cur_at[k // KB][:, k % KB, :])
                    for m in range(MT):
                        nc.tensor.matmul(yt_ps[m][:], xnt_slice(k, m), rhs,
                                         start=False, stop=True)
                        yts_sb = yts_pool.tile([P, NCHUNK], F32R, tag="yts")
                        nc.vector.tensor_copy(out=yts_sb[:], in_=yt_ps[m][:])
                        yts_list.append(yts_sb)
                else:
                    for m in range(MT):
                        yts_sb = yts_pool.tile([P, NCHUNK], F32R, tag="yts")
                        nc.vector.tensor_copy(out=yts_sb[:], in_=yt_ps[m][:])
                        yts_list.append(yts_sb)

                if ci == 0:
                    # chunk 0 epilogue runs immediately (PE would otherwise
                    # idle waiting for chunk 1's prefetched data)
                    emit_mlp()
                    cur_xch = alloc_xch()
                    for m in range(MT):
                        emit_tp(chunk_seq[0], m, cur_xch)
                    for m in range(MT):
                        emit_final_m(ch, m, yts_list, cur_xch)
                    pending = None
                else:
                    pending = (ch, yts_list, cur_xch)

                cur_at = nxt_at
                cur_xch = nxt_xch
            # tail: last chunk's finals
            if pending is not None:
                for m in range(MT):
                    emit_final_m(pending[0], m, pending[1], pending[2], tail=True)

    nc.finalize()
    return nc


_NC_CACHE = None


def _get_nc():
    global _NC_CACHE
    if _NC_CACHE is None:
        _NC_CACHE = build_kernel()
    return _NC_CACHE


def prep_in_maps(x, adj, st_emb, weights, bias, W1, b1, W2, b2):
    """Host-side layout prep -> per-core input dicts."""
    x = np.asarray(x, dtype=np.float32)
    adj = np.asarray(adj, dtype=np.float32)
    st_emb = np.asarray(st_emb, dtype=np.float32)
    weights = np.asarray(weights, dtype=np.float32)
    bias = np.asarray(bias, dtype=np.float32)
    W1 = np.asarray(W1, dtype=np.float32)
    b1 = np.asarray(b1, dtype=np.float32)
    W2 = np.asarray(W2, dtype=np.float32)
    b2 = np.asarray(b2, dtype=np.float32)

    adjT = np.ascontiguousarray(adj.T)
    sT = np.ascontiguousarray(st_emb.reshape(T, 64).T)          # [64, T]
    w0g, w1g = weights[0], weights[1]                            # [64, 64] each
    z = np.zeros((64, 64), np.float32)
    wd0 = np.block([[w0g, z], [z, w0g]])                         # [128, 128]
    wd1 = np.block([[w1g, z], [z, w1g]])
    biasp = np.concatenate([bias, bias]).reshape(P, 1)
    masks = np.zeros((2, P), np.float32)
    masks[0, :64] = 1.0
    masks[1, 64:] = 1.0
    b1p = b1.reshape(32, 1)
    b2p = b2.reshape(1, 1)

    shared = {
        "adjt": adjT, "sT": sT, "w1": W1, "b1p": b1p, "w2": W2, "b2p": b2p,
        "wd0": wd0, "wd1": wd1, "biasp": biasp, "masks": masks,
        "ident": np.eye(P, dtype=np.float32),
    }
    in_maps = []
    for b in range(B):
        xb = x[b]                                                # [T, N, C]
        xnode = np.ascontiguousarray(xb.transpose(1, 0, 2).reshape(N, TC))
        in_maps.append({"xnode": xnode, **shared})
    return in_maps


def assemble_output(results):
    """Per-core [TC, N] f32 -> full [B, T, N, C] f32."""
    outs = []
    for r in results:
        oc = r["out"].reshape(T, 64, N).transpose(0, 2, 1)       # [T, N, 64]
        outs.append(oc)
    return np.stack(outs, axis=0).astype(np.float32)


def run(inputs, **spmd_kwargs):
    nc = _get_nc()
    in_maps = prep_in_maps(**inputs)
    res = run_bass_kernel_spmd(nc, in_maps, core_ids=list(range(B)), **spmd_kwargs)
    return assemble_output(res.results), res


def kernel(**inputs) -> np.ndarray:
    out, _ = run(inputs)
    return out
